# revision 16
# baseline (speedup 1.0000x reference)
"""EnhancedGraphRegressor (9x GCNConv + 4x TransformerEncoder + pool/fc) on 8 trn2 cores.

Strategy: node/query sharding across 8 cores (512 rows each). The GCN scatter is
converted on host to a dense normalized-adjacency block A^T[:, core_block] that
stays SBUF-resident; each GCN layer is one 32-k-tile matmul chain + AllGather of
the updated node features. Attention runs flash-style over 32 key tiles with
per-head masked-Q score matmuls (PSUM), one fused exp (ACT, scale folded), and
col-tiled context accumulation with an extra ones-column producing the softmax
denominator. FFN/LayerNorm stay in the transposed [32, 512] per-core layout;
LayerNorm stats come from ones-vector matmuls, rsqrt via exp(-0.5*ln(v)) + one
Newton step (stays inside the exp/ln ACT table set).
"""
import sys

for _p in ('/opt/trn_rl_repo', '/opt/trn_rl_repo/concourse'):
    if _p not in sys.path:
        sys.path.insert(0, _p)

import numpy as np

N, EMB, HEADS, DH, NCONV, NDEC, FF = 4096, 32, 4, 8, 9, 4, 2048
NC, SBLK, P, KT = 8, 512, 128, 32
F32 = None  # set after imports


def _host_prep(inp):
    src, dst = np.asarray(inp["edge_index"][0]), np.asarray(inp["edge_index"][1])
    loops = np.arange(N, dtype=src.dtype)
    srcf = np.concatenate([src, loops])
    dstf = np.concatenate([dst, loops])
    deg = np.bincount(dstf, minlength=N).astype(np.float32)
    dinv = 1.0 / np.sqrt(np.maximum(deg, 1.0))
    w = (dinv[srcf] * dinv[dstf]).astype(np.float32)
    # AT3[c, src, dst_local]: per-core A^T column blocks, already stacked for shard_map
    AT3 = np.zeros((NC, N, SBLK), np.float32)
    np.add.at(AT3, (dstf // SBLK, srcf, dstf % SBLK), w)
    a1 = np.bincount(dstf, weights=w.astype(np.float64), minlength=N).astype(np.float32)

    pre = {"AT3": AT3, "a1": a1}
    w3 = np.zeros((3, EMB), np.float32)
    w3[0:2] = inp["embed_w"].T
    w3[2] = inp["embed_b"]
    pre["w3"] = w3
    gw = np.zeros((NCONV, 33, EMB), np.float32)
    for i in range(NCONV):
        gw[i, 0:32] = inp["conv_w"][i].T
        gw[i, 32] = inp["conv_b"][i]
    pre["gw"] = gw
    qw = np.zeros((NDEC, 33, 128), np.float32)
    kw = np.zeros((NDEC, 33, 128), np.float32)
    vw = np.zeros((NDEC, 33, 36), np.float32)
    wo = np.zeros((NDEC, 128, 32), np.float32)
    for l in range(NDEC):
        W, b = np.asarray(inp["qkv_w"][l]), np.asarray(inp["qkv_b"][l])
        for h in range(HEADS):
            for d in range(DH):
                qw[l, 0:32, 32 * h + d] = W[8 * h + d]
                qw[l, 32, 32 * h + d] = b[8 * h + d]
                kw[l, 0:32, 32 * h + d] = W[32 + 8 * h + d]
                kw[l, 32, 32 * h + d] = b[32 + 8 * h + d]
                vw[l, 0:32, 9 * h + d] = W[64 + 8 * h + d]
                vw[l, 32, 9 * h + d] = b[64 + 8 * h + d]
            vw[l, 32, 9 * h + 8] = 1.0   # ones column -> softmax denominator
            wo[l, 32 * h:32 * h + 8] = np.asarray(inp["out_w"][l])[:, 8 * h:8 * h + 8].T
        wo[l, 8] += inp["out_b"][l]
    pre.update(qw=qw, kw=kw, vw=vw, wo=wo)
    E128 = np.zeros((128, 128), np.float32)
    for h in range(HEADS):
        E128[32 * h + 8, 32 * h:32 * h + 32] = 1.0
    pre["E128"] = E128
    f1 = np.zeros((NDEC, 33, FF), np.float32)
    for l in range(NDEC):
        f1[l, 0:32] = inp["ff1_w"][l].T
        f1[l, 32] = inp["ff1_b"][l]
    pre["f1"] = f1
    pre["f2"] = np.ascontiguousarray(np.transpose(np.asarray(inp["ff2_w"]), (0, 2, 1)))
    pre["f2b"] = np.asarray(inp["ff2_b"], np.float32)
    lnw = np.stack([inp["ln1_w"], inp["ln1_b"], inp["ln2_w"], inp["ln2_b"]], 0)
    pre["lnw"] = np.ascontiguousarray(np.transpose(np.asarray(lnw, np.float32), (2, 0, 1)))  # [32, 4, NDEC]
    fca = np.zeros((33, 2), np.float32)
    fca[0:32] = inp["fc_w"].T
    fca[32] = inp["fc_b"]
    pre["fca"] = fca
    return pre


def _build(nc, tc, tile, mybir, bass, make_identity):
    import os
    STAGE = int(os.environ.get("KSTAGE", "99"))
    F32 = mybir.dt.float32
    AF = mybir.ActivationFunctionType
    ALU = mybir.AluOpType
    RG = [list(range(NC))]
    SCALE = float(1.0 / np.sqrt(DH))

    # ---- DRAM I/O ----
    d_at = nc.dram_tensor("a_t", [N, SBLK], F32, kind="ExternalInput")
    d_a1 = nc.dram_tensor("a1", [SBLK], F32, kind="ExternalInput")
    d_x = nc.dram_tensor("x", [N, 2], F32, kind="ExternalInput")
    d_w3 = nc.dram_tensor("w3", [3, EMB], F32, kind="ExternalInput")
    d_gw = nc.dram_tensor("gw", [NCONV, 33, EMB], F32, kind="ExternalInput")
    d_qw = nc.dram_tensor("qw", [NDEC, 33, 128], F32, kind="ExternalInput")
    d_kw = nc.dram_tensor("kw", [NDEC, 33, 128], F32, kind="ExternalInput")
    d_vw = nc.dram_tensor("vw", [NDEC, 33, 36], F32, kind="ExternalInput")
    d_wo = nc.dram_tensor("wo", [NDEC, 128, 32], F32, kind="ExternalInput")
    d_e128 = nc.dram_tensor("e128", [128, 128], F32, kind="ExternalInput")
    d_f1 = nc.dram_tensor("f1", [NDEC, 33, FF], F32, kind="ExternalInput")
    d_f2 = nc.dram_tensor("f2", [NDEC, FF, EMB], F32, kind="ExternalInput")
    d_f2b = nc.dram_tensor("f2b", [NDEC, EMB], F32, kind="ExternalInput")
    d_lnw = nc.dram_tensor("lnw", [EMB, 4, NDEC], F32, kind="ExternalInput")
    d_fca = nc.dram_tensor("fca", [33, 2], F32, kind="ExternalInput")
    d_out = nc.dram_tensor("out", [1, 2], F32, kind="ExternalOutput")
    if os.environ.get("KDBG") == "1":
        d_dbgA = nc.dram_tensor("dbgA", [128, 4096], F32, kind="ExternalOutput")
        d_dbgB = nc.dram_tensor("dbgB", [33, 4096], F32, kind="ExternalOutput")

    from contextlib import ExitStack
    es = ExitStack()
    cp = es.enter_context(tc.tile_pool(name="const", bufs=1))
    wp = es.enter_context(tc.tile_pool(name="work", bufs=1))
    ep = es.enter_context(tc.tile_pool(name="exp", bufs=3))
    gp = es.enter_context(tc.tile_pool(name="gwork", bufs=3))
    ffp = es.enter_context(tc.tile_pool(name="ffw", bufs=2))
    ps_sc = es.enter_context(tc.tile_pool(name="ps_sc", bufs=4, space="PSUM"))
    ps_g = es.enter_context(tc.tile_pool(name="ps_g", bufs=2, space="PSUM"))
    ps_ctx = es.enter_context(tc.tile_pool(name="ps_ctx", bufs=1, space="PSUM"))
    ps_s = es.enter_context(tc.tile_pool(name="ps_s", bufs=1, space="PSUM"))
    dp = es.enter_context(tc.tile_pool(name="dram", bufs=2, space="DRAM"))

    # ---- persistent SBUF ----
    At = cp.tile([P, KT, SBLK], F32)
    xs = cp.tile([P, KT, 2], F32)
    hN = cp.tile([P, KT, EMB], F32)
    hTfull = cp.tile([33, N], F32)
    hTown = cp.tile([33, SBLK], F32)
    U_aug = cp.tile([33, SBLK], F32)
    U0_aug = cp.tile([3, SBLK], F32)
    x2_aug = cp.tile([33, SBLK], F32)
    Karr = cp.tile([P, N], F32)
    Varr = cp.tile([P, KT, 36], F32)
    Qm = cp.tile([P, HEADS, SBLK], F32)
    w3t = cp.tile([3, EMB], F32)
    gwt = cp.tile([33, NCONV, EMB], F32)
    qwt = cp.tile([33, NDEC, 128], F32)
    kwt = cp.tile([33, NDEC, 128], F32)
    vwt = cp.tile([33, NDEC, 36], F32)
    wot = cp.tile([P, NDEC, 32], F32)
    e128t = cp.tile([P, 128], F32)
    f2bt = cp.tile([1, NDEC, EMB], F32)
    lnwt = cp.tile([EMB, 4, NDEC], F32)
    fcat = cp.tile([33, 2], F32)
    ident32 = cp.tile([32, 32], F32)
    ones32inv = cp.tile([32, 1], F32)
    ones1_32 = cp.tile([1, 32], F32)
    ones_row = cp.tile([1, SBLK], F32)
    epsA = cp.tile([1, 1], F32)

    # ---- stage 0: loads + const init ----
    for kt in range(KT):
        nc.sync.dma_start(out=At[:, kt, :], in_=d_at.ap()[P * kt:P * (kt + 1), :])
    nc.sync.dma_start(out=xs[:], in_=d_x.ap().rearrange("(k p) e -> p k e", k=KT))
    nc.sync.dma_start(out=U0_aug[2:3, :], in_=d_a1.ap()[None, :])
    nc.sync.dma_start(out=w3t[:], in_=d_w3.ap())
    nc.sync.dma_start(out=gwt[:], in_=d_gw.ap().rearrange("i r e -> r i e"))
    nc.sync.dma_start(out=qwt[:], in_=d_qw.ap().rearrange("l r e -> r l e"))
    nc.sync.dma_start(out=kwt[:], in_=d_kw.ap().rearrange("l r e -> r l e"))
    nc.sync.dma_start(out=vwt[:], in_=d_vw.ap().rearrange("l r e -> r l e"))
    nc.sync.dma_start(out=wot[:], in_=d_wo.ap().rearrange("l r e -> r l e"))
    nc.sync.dma_start(out=e128t[:], in_=d_e128.ap())
    nc.sync.dma_start(out=f2bt[:], in_=d_f2b.ap()[None, :, :])
    nc.sync.dma_start(out=lnwt[:], in_=d_lnw.ap())
    nc.sync.dma_start(out=fcat[:], in_=d_fca.ap())
    make_identity(nc, ident32[:])
    nc.vector.memset(ones32inv[:], 1.0 / 32.0)
    nc.vector.memset(ones1_32[:], 1.0)
    nc.vector.memset(ones_row[:], 1.0)
    nc.vector.memset(epsA[:], 1e-5)
    nc.vector.memset(U_aug[32:33, :], 1.0)
    nc.vector.memset(x2_aug[32:33, :], 1.0)
    nc.vector.memset(hTown[32:33, :], 1.0)
    nc.vector.memset(hTfull[32:33, :], 1.0)
    nc.vector.memset(Qm[:], 0.0)

    def ag_normal():
        """hTown[0:32] -> 4 transposes -> AG -> hN full."""
        hNo = wp.tile([P, 4, EMB], F32, tag="hNo", bufs=2)
        for k in range(4):
            tp = ps_s.tile([P, EMB], F32, tag="s")
            nc.tensor.transpose(tp[:], hTown[0:32, P * k:P * (k + 1)], ident32[:])
            nc.vector.tensor_copy(hNo[:, k, :], tp[:])
        agi = dp.tile([SBLK, EMB], F32, tag="agNi")
        ago = dp.tile([N, EMB], F32, tag="agNo")
        nc.sync.dma_start(out=agi[:].rearrange("(k p) e -> p k e", k=4), in_=hNo[:])
        nc.gpsimd.collective_compute("AllGather", mybir.AluOpType.bypass,
                                     replica_groups=RG, ins=[agi.opt()], outs=[ago.opt()])
        agov = ago[:].rearrange("(k p) e -> p k e", k=KT)
        for g in range(4):
            nc.sync.dma_start(out=hN[:, 8 * g:8 * (g + 1), :], in_=agov[:, 8 * g:8 * (g + 1), :])

    def ag_transposed():
        """hTown[0:32] -> AG -> hTfull[0:32]."""
        agi = dp.tile([32, SBLK], F32, tag="agTi")
        ago = dp.tile([NC * 32, SBLK], F32, tag="agTo")
        nc.sync.dma_start(out=agi[:], in_=hTown[0:32, :])
        nc.gpsimd.collective_compute("AllGather", mybir.AluOpType.bypass,
                                     replica_groups=RG, ins=[agi.opt()], outs=[ago.opt()])
        srcv = ago[:].rearrange("(c e) s -> e c s", c=NC)
        dstv = hTfull[0:32, :].rearrange("e (c s) -> e c s", c=NC)
        for g in range(2):
            nc.sync.dma_start(out=dstv[:, 4 * g:4 * (g + 1), :], in_=srcv[:, 4 * g:4 * (g + 1), :])

    # ---- GCN layer 1 (embed folded) ----
    p0 = ps_s.tile([2, SBLK], F32, tag="s")
    for kt in range(KT):
        nc.tensor.matmul(p0[:], xs[:, kt, :], At[:, kt, :], start=(kt == 0), stop=(kt == KT - 1))
    nc.vector.tensor_copy(U0_aug[0:2, :], p0[:])
    u1 = ps_s.tile([EMB, SBLK], F32, tag="s")
    nc.tensor.matmul(u1[:], w3t[:], U0_aug[:], start=True, stop=True)
    nc.vector.tensor_copy(U_aug[0:32, :], u1[:])
    z1 = ps_s.tile([EMB, SBLK], F32, tag="s")
    nc.tensor.matmul(z1[:], gwt[:, 0, :], U_aug[:], start=True, stop=True)
    nc.scalar.activation(hTown[0:32, :], z1[:], AF.Relu)
    ag_normal()
    if STAGE == 1:
        nc.sync.dma_start(out=d_dbgA.ap()[:, 0:KT * EMB],
                          in_=hN[:].rearrange("p k e -> p (k e)"))
        es.close()
        return

    # ---- GCN layers 2..9 ----
    for i in range(1, NCONV):
        u = ps_s.tile([EMB, SBLK], F32, tag="s")
        for kt in range(KT):
            nc.tensor.matmul(u[:], hN[:, kt, :], At[:, kt, :], start=(kt == 0), stop=(kt == KT - 1))
        nc.vector.tensor_copy(U_aug[0:32, :], u[:])
        z = ps_s.tile([EMB, SBLK], F32, tag="s")
        nc.tensor.matmul(z[:], gwt[:, i, :], U_aug[:], start=True, stop=True)
        nc.scalar.activation(hTown[0:32, :], z[:], AF.Relu)
        if i < NCONV - 1:
            ag_normal()
        else:
            ag_transposed()
    if STAGE == 2:
        if os.environ.get("KDBG") == "1":
            nc.sync.dma_start(out=d_dbgB.ap(), in_=hTfull[:])
        es.close()
        return

    # ---- LayerNorm helper (transposed layout) ----
    def layer_norm(res_psum, add_sbuf, w_ap, b_ap, out_ap):
        xsq = wp.tile([32, 2 * SBLK], F32, tag="xsq")
        nc.vector.tensor_add(xsq[:, 0:SBLK], res_psum, add_sbuf)
        nc.vector.tensor_mul(xsq[:, SBLK:], xsq[:, 0:SBLK], xsq[:, 0:SBLK])
        stats = wp.tile([1, 2 * SBLK], F32, tag="stats")
        st_a = ps_s.tile([1, SBLK], F32, tag="s")
        nc.tensor.matmul(st_a[:], ones32inv[:], xsq[:, 0:SBLK], start=True, stop=True)
        nc.vector.tensor_copy(stats[:, 0:SBLK], st_a[:])
        st_b = ps_s.tile([1, SBLK], F32, tag="s")
        nc.tensor.matmul(st_b[:], ones32inv[:], xsq[:, SBLK:], start=True, stop=True)
        nc.vector.tensor_copy(stats[:, SBLK:], st_b[:])
        veps = wp.tile([1, SBLK], F32, tag="veps")
        # veps = meansq - mean^2 + eps
        m2 = wp.tile([1, SBLK], F32, tag="m2")
        nc.vector.tensor_mul(m2[:], stats[:, 0:SBLK], stats[:, 0:SBLK])
        nc.vector.tensor_sub(veps[:], stats[:, SBLK:], m2[:])
        lnv = wp.tile([1, SBLK], F32, tag="lnv")
        nc.scalar.activation(lnv[:], veps[:], AF.Ln, bias=epsA[:])
        y0 = wp.tile([1, SBLK], F32, tag="y0")
        nc.scalar.activation(y0[:], lnv[:], AF.Exp, scale=-0.5)
        # one Newton step: y1 = y0 * (1.5 - 0.5 * veps * y0^2)
        nt = wp.tile([1, SBLK], F32, tag="nt")
        nc.vector.tensor_mul(nt[:], y0[:], y0[:])
        nc.vector.tensor_mul(nt[:], nt[:], veps[:])
        nc.vector.tensor_scalar(nt[:], nt[:], -0.5, 1.5, mybir.AluOpType.mult, mybir.AluOpType.add)
        iq = wp.tile([1, 2 * SBLK], F32, tag="iq")
        nc.vector.tensor_mul(iq[:, 0:SBLK], y0[:], nt[:])
        nc.vector.tensor_mul(iq[:, SBLK:], stats[:, 0:SBLK], iq[:, 0:SBLK])
        rep2a = ps_s.tile([32, SBLK], F32, tag="s")
        nc.tensor.matmul(rep2a[:], ones1_32[:], iq[:, 0:SBLK], start=True, stop=True)
        t1 = wp.tile([32, SBLK], F32, tag="t1")
        nc.vector.tensor_mul(t1[:], xsq[:, 0:SBLK], rep2a[:])
        rep2b = ps_s.tile([32, SBLK], F32, tag="s")
        nc.tensor.matmul(rep2b[:], ones1_32[:], iq[:, SBLK:], start=True, stop=True)
        nc.vector.tensor_sub(t1[:], t1[:], rep2b[:])
        nc.vector.tensor_scalar(out_ap, t1[:], w_ap, b_ap, mybir.AluOpType.mult, mybir.AluOpType.add)

    # ---- transformer layers ----
    for l in range(NDEC):
        # K_arr
        for j in range(8):
            pk = ps_g.tile([P, SBLK], F32, tag="pg")
            nc.tensor.matmul(pk[:], kwt[:, l, :], hTfull[:, SBLK * j:SBLK * (j + 1)], start=True, stop=True)
            nc.vector.tensor_copy(Karr[:, SBLK * j:SBLK * (j + 1)], pk[:])
        # Q + masked per-head copies
        pq = ps_g.tile([P, SBLK], F32, tag="pg")
        nc.tensor.matmul(pq[:], qwt[:, l, :], hTown[:], start=True, stop=True)
        for h in range(HEADS):
            nc.vector.tensor_copy(Qm[32 * h:32 * h + 8, h, :], pq[32 * h:32 * h + 8, :])
        # V_arr (ones column generated via vw aug row)
        for g in range(KT // 4):
            pv = ps_s.tile([P, 4, 36], F32, tag="s")
            for q in range(4):
                kt = 4 * g + q
                nc.tensor.matmul(pv[:, q, :], hTfull[:, P * kt:P * (kt + 1)], vwt[:, l, :],
                                 start=True, stop=True)
            nc.vector.tensor_copy(Varr[:, 4 * g:4 * (g + 1), :], pv[:])
        if STAGE == 3 and l == 0:
            nc.sync.dma_start(out=d_dbgA.ap(), in_=Karr[:])
            nc.sync.dma_start(out=d_dbgB.ap()[0:33, 0:SBLK], in_=hTown[:])
            es.close()
            return
        # flash loop
        ctx = ps_ctx.tile([P, SBLK], F32, tag="ctx")
        nc.vector.memset(ctx[:], 0.0)
        for kt in range(KT):
            for h in range(HEADS):
                S = ps_sc.tile([P, SBLK], F32, tag="S", bufs=4)
                nc.tensor.matmul(S[:], Karr[:, P * kt:P * (kt + 1)], Qm[:, h, :],
                                 start=True, stop=True)
                E = ep.tile([P, SBLK], F32, tag="E", bufs=6)
                nc.scalar.activation(E[:], S[:], AF.Exp, scale=SCALE)
                nc.tensor.matmul(ctx[32 * h:32 * h + 9, :], Varr[:, kt, 9 * h:9 * h + 9],
                                 E[:],
                                 start=(kt == 0), stop=(kt == KT - 1),
                                 tile_position=(0, 32 * h))
        if STAGE == 4 and l == 0:
            ctd = gp.tile([P, SBLK], F32, tag="ctd", bufs=1)
            nc.vector.tensor_copy(ctd[:], ctx[:])
            nc.sync.dma_start(out=d_dbgA.ap()[:, 0:SBLK], in_=ctd[:])
            es.close()
            return
        # softmax denominators + out-projection
        cte = gp.tile([P, SBLK], F32, tag="cte", bufs=1)
        nc.vector.tensor_scalar(cte[:], ctx[:], 1e-30, None, mybir.AluOpType.add)
        rcp = gp.tile([P, SBLK], F32, tag="rcp", bufs=1)
        nc.vector.reciprocal(rcp[:], cte[:])
        rep = ps_g.tile([P, SBLK], F32, tag="pg")
        nc.tensor.matmul(rep[:], e128t[:], rcp[:], start=True, stop=True)
        ctn = gp.tile([P, SBLK], F32, tag="ctn", bufs=1)
        nc.vector.tensor_mul(ctn[:], cte[:], rep[:])
        attn = ps_s.tile([32, SBLK], F32, tag="s")
        nc.tensor.matmul(attn[:], wot[:, l, :], ctn[:], start=True, stop=True)
        # LN1 -> x2_aug
        layer_norm(attn[:], hTown[0:32, :], lnwt[:, 0, l:l + 1], lnwt[:, 1, l:l + 1],
                   x2_aug[0:32, :])
        if STAGE == 5 and l == 0:
            nc.sync.dma_start(out=d_dbgB.ap()[0:33, 0:SBLK], in_=x2_aug[:])
            es.close()
            return
        # FFN
        f1t = ffp.tile([33, FF], F32, tag="f1")
        nc.sync.dma_start(out=f1t[:], in_=d_f1.ap()[l])
        f2t = ffp.tile([P, FF // P, EMB], F32, tag="f2")
        nc.sync.dma_start(out=f2t[:], in_=d_f2.ap()[l].rearrange("(t p) e -> p t e", p=P))
        y = ps_s.tile([EMB, SBLK], F32, tag="s")
        for ft in range(FF // P):
            g_ps = ps_g.tile([P, SBLK], F32, tag="pg")
            nc.tensor.matmul(g_ps[:], f1t[:, P * ft:P * (ft + 1)], x2_aug[:], start=True, stop=True)
            g_sb = gp.tile([P, SBLK], F32, tag="g")
            if ft % 2 == 0:
                nc.scalar.activation(g_sb[:], g_ps[:], AF.Relu)
            else:
                nc.vector.tensor_scalar(g_sb[:], g_ps[:], 0.0, None, mybir.AluOpType.max)
            nc.tensor.matmul(y[:], f2t[:, ft, :], g_sb[:], start=(ft == 0), stop=False)
        nc.tensor.matmul(y[:], f2bt[:, l, :], ones_row[:], start=False, stop=True)
        # LN2 -> hTown
        layer_norm(y[:], x2_aug[0:32, :], lnwt[:, 2, l:l + 1], lnwt[:, 3, l:l + 1],
                   hTown[0:32, :])
        if STAGE == 6 and l == 0:
            nc.sync.dma_start(out=d_dbgB.ap()[0:33, 0:SBLK], in_=hTown[:])
            es.close()
            return
        if l < NDEC - 1:
            ag_transposed()

    # ---- pooling + fc ----
    red = wp.tile([32, 1], F32, tag="red")
    nc.vector.reduce_sum(red[:], hTown[0:32, :], axis=mybir.AxisListType.X)
    po = wp.tile([32, 1], F32, tag="po")
    nc.vector.tensor_scalar(po[:], red[:], 1.0 / N, None, mybir.AluOpType.mult)
    agi = dp.tile([32, 1], F32, tag="agPi")
    ago = dp.tile([NC * 32, 1], F32, tag="agPo")
    nc.sync.dma_start(out=agi[:], in_=po[:])
    nc.gpsimd.collective_compute("AllGather", mybir.AluOpType.bypass,
                                 replica_groups=RG, ins=[agi.opt()], outs=[ago.opt()])
    pool8 = wp.tile([32, NC], F32, tag="pool8")
    nc.sync.dma_start(out=pool8[:], in_=ago[:].rearrange("(c e) o -> e (c o)", c=NC))
    pa = cp.tile([33, 1], F32)
    nc.vector.memset(pa[32:33, :], 1.0)
    nc.vector.reduce_sum(pa[0:32, :], pool8[:], axis=mybir.AxisListType.X)
    op = ps_s.tile([1, 2], F32, tag="s")
    nc.tensor.matmul(op[:], pa[:], fcat[:], start=True, stop=True)
    osb = wp.tile([1, 2], F32, tag="osb")
    nc.vector.tensor_copy(osb[:], op[:])
    nc.sync.dma_start(out=d_out.ap(), in_=osb[:])
    es.close()


_CACHE = {}


def _get_program():
    import os
    key = "nc" + os.environ.get("KSTAGE", "99") + os.environ.get("KDBG", "0")
    if key in _CACHE:
        return _CACHE[key]
    import concourse.bass as bass
    import concourse.mybir as mybir
    import concourse.tile as tile
    from concourse import bacc
    from concourse.masks import make_identity

    nc = bacc.Bacc("TRN2", target_bir_lowering=False, debug=False, num_devices=NC)
    with tile.TileContext(nc) as tc:
        _build(nc, tc, tile, mybir, bass, make_identity)
    nc.compile()
    _CACHE[key] = nc
    return nc


def _get_runner():
    """Cached shard_map executable over 8 cores (modeled on run_bass_via_pjrt)."""
    if "runner" in _CACHE:
        return _CACHE["runner"]
    import jax
    globals()["jax"] = jax
    import concourse.mybir as mybir
    from concourse import bass2jax

    nc = _get_program()
    bass2jax.install_neuronx_cc_hook()

    part_name = nc.partition_id_tensor.name if nc.partition_id_tensor else None
    in_names, out_names, out_avals, zero_outs = [], [], [], []
    for alloc in nc.m.functions[0].allocations:
        if not isinstance(alloc, mybir.MemoryLocationSet):
            continue
        name = alloc.memorylocations[0].name
        if alloc.kind == "ExternalInput":
            if name != part_name:
                in_names.append(name)
        elif alloc.kind == "ExternalOutput":
            shape = tuple(alloc.tensor_shape)
            dtype = mybir.dt.np(alloc.dtype)
            out_names.append(name)
            out_avals.append(jax.core.ShapedArray(shape, dtype))
            zero_outs.append(np.zeros(shape, dtype))
    n_params = len(in_names)
    all_names = in_names + out_names
    if part_name is not None:
        all_names = all_names + [part_name]

    def _body(*args):
        operands = list(args)
        if part_name is not None:
            operands.append(bass2jax.partition_id_tensor())
        outs = bass2jax._bass_exec_p.bind(
            *operands,
            out_avals=tuple(out_avals),
            in_names=tuple(all_names),
            out_names=tuple(out_names),
            lowering_input_output_aliases=(),
            sim_require_finite=True,
            sim_require_nnan=True,
            nc=nc,
        )
        return tuple(outs)

    devices = jax.devices()[:NC]
    mesh = bass2jax.Mesh(np.asarray(devices), ("core",))
    n_outs = len(out_names)
    sharded = jax.jit(
        bass2jax.shard_map(
            _body, mesh=mesh,
            in_specs=(bass2jax.PartitionSpec("core"),) * (n_params + n_outs),
            out_specs=(bass2jax.PartitionSpec("core"),) * n_outs,
            check_rep=False,
        ),
        donate_argnums=tuple(range(n_params, n_params + n_outs)),
        keep_unused=True,
    )

    from jax.sharding import NamedSharding, PartitionSpec as PS
    shard = NamedSharding(mesh, PS("core"))

    def _stage(shared, per_core, dev_key):
        concat_in = []
        for nm in in_names:
            if nm in per_core:
                concat_in.append(np.ascontiguousarray(per_core[nm]))
            else:
                a = np.ascontiguousarray(shared[nm])
                concat_in.append(np.broadcast_to(a, (NC, *a.shape)).reshape(NC * a.shape[0], *a.shape[1:]))
        dev_arrs = [jax.device_put(a, shard) for a in concat_in]
        for a in dev_arrs:
            a.block_until_ready()
        dev = (dev_key, dev_arrs)
        _CACHE["dev_in"] = dev
        return dev

    def run(shared, per_core):
        import time as _time
        dev_key = ("dev", id(shared), id(per_core))
        dev = _CACHE.get("dev_in")
        if dev is None or dev[0] != dev_key:
            dev = _stage(shared, per_core, dev_key)
        last_exc = None
        for attempt in range(5):
            try:
                concat_zeros = [np.zeros((NC * z.shape[0], *z.shape[1:]), z.dtype) for z in zero_outs]
                out_arrs = sharded(*dev[1], *concat_zeros)
                return {
                    nm: np.asarray(out_arrs[i]).reshape(NC, *out_avals[i].shape)[0]
                    for i, nm in enumerate(out_names)
                }
            except Exception as e:  # transient device-unrecoverable after aborted runs
                last_exc = e
                _time.sleep(4.0 * (attempt + 1))
                dev = _stage(shared, per_core, dev_key)
        raise last_exc

    _CACHE["runner"] = run
    _CACHE["sharded_fn"] = sharded
    return run


def _input_key(inp):
    import hashlib
    hsh = hashlib.sha256()
    for k in sorted(inp):
        hsh.update(k.encode())
        hsh.update(np.ascontiguousarray(inp[k]).tobytes())
    return hsh.hexdigest()


def kernel(**inputs):
    inp = {k: np.asarray(v) for k, v in inputs.items()}
    key = _input_key(inp)
    run = _get_runner()
    cached = _CACHE.get("staged")
    if cached is None or cached[0] != key:
        pre = _host_prep(inp)
        shared = {
            "x": np.ascontiguousarray(inp["x"], np.float32),
            "w3": pre["w3"], "gw": pre["gw"], "qw": pre["qw"], "kw": pre["kw"],
            "vw": pre["vw"], "wo": pre["wo"], "e128": pre["E128"], "f1": pre["f1"],
            "f2": pre["f2"], "f2b": pre["f2b"], "lnw": pre["lnw"], "fca": pre["fca"],
        }
        per_core = {
            "a_t": pre["AT3"].reshape(NC * N, SBLK),
            "a1": pre["a1"].reshape(NC, SBLK).reshape(NC * SBLK),
        }
        _CACHE["staged"] = (key, shared, per_core)
    else:
        _, shared, per_core = cached

    outs = run(shared, per_core)
    kernel.last_outs = outs
    return outs["out"]


if __name__ == "__main__":
    import test as T
    T.main()


# revision 17
# speedup vs baseline: 1.2133x; 1.2133x over previous
"""EnhancedGraphRegressor (9x GCNConv + 4x TransformerEncoder + pool/fc) on 8 trn2 cores.

Strategy: node/query sharding across 8 cores (512 rows each). The GCN scatter is
converted on host to a dense normalized-adjacency block A^T[:, core_block] that
stays SBUF-resident; each GCN layer is one 32-k-tile matmul chain + AllGather of
the updated node features. Attention runs flash-style over 32 key tiles with
per-head masked-Q score matmuls (PSUM), one fused exp (ACT, scale folded), and
col-tiled context accumulation with an extra ones-column producing the softmax
denominator. FFN/LayerNorm stay in the transposed [32, 512] per-core layout;
LayerNorm stats come from ones-vector matmuls, rsqrt via exp(-0.5*ln(v)) + one
Newton step (stays inside the exp/ln ACT table set).
"""
import sys

for _p in ('/opt/trn_rl_repo', '/opt/trn_rl_repo/concourse'):
    if _p not in sys.path:
        sys.path.insert(0, _p)

import numpy as np

N, EMB, HEADS, DH, NCONV, NDEC, FF = 4096, 32, 4, 8, 9, 4, 2048
NC, SBLK, P, KT = 8, 512, 128, 32
F32 = None  # set after imports


def _host_prep(inp):
    src, dst = np.asarray(inp["edge_index"][0]), np.asarray(inp["edge_index"][1])
    loops = np.arange(N, dtype=src.dtype)
    srcf = np.concatenate([src, loops])
    dstf = np.concatenate([dst, loops])
    deg = np.bincount(dstf, minlength=N).astype(np.float32)
    dinv = 1.0 / np.sqrt(np.maximum(deg, 1.0))
    w = (dinv[srcf] * dinv[dstf]).astype(np.float32)
    # AT3[c, src, dst_local]: per-core A^T column blocks, already stacked for shard_map
    AT3 = np.zeros((NC, N, SBLK), np.float32)
    np.add.at(AT3, (dstf // SBLK, srcf, dstf % SBLK), w)
    a1 = np.bincount(dstf, weights=w.astype(np.float64), minlength=N).astype(np.float32)

    pre = {"AT3": AT3, "a1": a1}
    w3 = np.zeros((3, EMB), np.float32)
    w3[0:2] = inp["embed_w"].T
    w3[2] = inp["embed_b"]
    pre["w3"] = w3
    gw = np.zeros((NCONV, 33, EMB), np.float32)
    for i in range(NCONV):
        gw[i, 0:32] = inp["conv_w"][i].T
        gw[i, 32] = inp["conv_b"][i]
    pre["gw"] = gw
    qw = np.zeros((NDEC, 33, 128), np.float32)
    kw = np.zeros((NDEC, 33, 128), np.float32)
    vw = np.zeros((NDEC, 33, 36), np.float32)
    wo = np.zeros((NDEC, 128, 32), np.float32)
    for l in range(NDEC):
        W, b = np.asarray(inp["qkv_w"][l]), np.asarray(inp["qkv_b"][l])
        for h in range(HEADS):
            for d in range(DH):
                qw[l, 0:32, 32 * h + d] = W[8 * h + d]
                qw[l, 32, 32 * h + d] = b[8 * h + d]
                kw[l, 0:32, 32 * h + d] = W[32 + 8 * h + d]
                kw[l, 32, 32 * h + d] = b[32 + 8 * h + d]
                vw[l, 0:32, 9 * h + d] = W[64 + 8 * h + d]
                vw[l, 32, 9 * h + d] = b[64 + 8 * h + d]
            vw[l, 32, 9 * h + 8] = 1.0   # ones column -> softmax denominator
            wo[l, 32 * h:32 * h + 8] = np.asarray(inp["out_w"][l])[:, 8 * h:8 * h + 8].T
        wo[l, 8] += inp["out_b"][l]
    pre.update(qw=qw, kw=kw, vw=vw, wo=wo)
    E128 = np.zeros((128, 128), np.float32)
    for h in range(HEADS):
        E128[32 * h + 8, 32 * h:32 * h + 32] = 1.0
    pre["E128"] = E128
    f1 = np.zeros((NDEC, 33, FF), np.float32)
    for l in range(NDEC):
        f1[l, 0:32] = inp["ff1_w"][l].T
        f1[l, 32] = inp["ff1_b"][l]
    pre["f1"] = f1
    pre["f2"] = np.ascontiguousarray(np.transpose(np.asarray(inp["ff2_w"]), (0, 2, 1)))
    pre["f2b"] = np.asarray(inp["ff2_b"], np.float32)
    lnw = np.stack([inp["ln1_w"], inp["ln1_b"], inp["ln2_w"], inp["ln2_b"]], 0)
    pre["lnw"] = np.ascontiguousarray(np.transpose(np.asarray(lnw, np.float32), (2, 0, 1)))  # [32, 4, NDEC]
    fca = np.zeros((33, 2), np.float32)
    fca[0:32] = inp["fc_w"].T
    fca[32] = inp["fc_b"]
    pre["fca"] = fca
    return pre


def _build(nc, tc, tile, mybir, bass, make_identity):
    import os
    STAGE = int(os.environ.get("KSTAGE", "99"))
    F32 = mybir.dt.float32
    AF = mybir.ActivationFunctionType
    ALU = mybir.AluOpType
    RG = [list(range(NC))]
    SCALE = float(1.0 / np.sqrt(DH))

    # ---- DRAM I/O ----
    d_at = nc.dram_tensor("a_t", [N, SBLK], F32, kind="ExternalInput")
    d_a1 = nc.dram_tensor("a1", [SBLK], F32, kind="ExternalInput")
    d_x = nc.dram_tensor("x", [N, 2], F32, kind="ExternalInput")
    d_w3 = nc.dram_tensor("w3", [3, EMB], F32, kind="ExternalInput")
    d_gw = nc.dram_tensor("gw", [NCONV, 33, EMB], F32, kind="ExternalInput")
    d_qw = nc.dram_tensor("qw", [NDEC, 33, 128], F32, kind="ExternalInput")
    d_kw = nc.dram_tensor("kw", [NDEC, 33, 128], F32, kind="ExternalInput")
    d_vw = nc.dram_tensor("vw", [NDEC, 33, 36], F32, kind="ExternalInput")
    d_wo = nc.dram_tensor("wo", [NDEC, 128, 32], F32, kind="ExternalInput")
    d_e128 = nc.dram_tensor("e128", [128, 128], F32, kind="ExternalInput")
    d_f1 = nc.dram_tensor("f1", [NDEC, 33, FF], F32, kind="ExternalInput")
    d_f2 = nc.dram_tensor("f2", [NDEC, FF, EMB], F32, kind="ExternalInput")
    d_f2b = nc.dram_tensor("f2b", [NDEC, EMB], F32, kind="ExternalInput")
    d_lnw = nc.dram_tensor("lnw", [EMB, 4, NDEC], F32, kind="ExternalInput")
    d_fca = nc.dram_tensor("fca", [33, 2], F32, kind="ExternalInput")
    d_out = nc.dram_tensor("out", [1, 2], F32, kind="ExternalOutput")
    if os.environ.get("KDBG") == "1":
        d_dbgA = nc.dram_tensor("dbgA", [128, 4096], F32, kind="ExternalOutput")
        d_dbgB = nc.dram_tensor("dbgB", [33, 4096], F32, kind="ExternalOutput")

    from contextlib import ExitStack
    es = ExitStack()
    cp = es.enter_context(tc.tile_pool(name="const", bufs=1))
    wp = es.enter_context(tc.tile_pool(name="work", bufs=1))
    ep = es.enter_context(tc.tile_pool(name="exp", bufs=3))
    gp = es.enter_context(tc.tile_pool(name="gwork", bufs=3))
    ffp = es.enter_context(tc.tile_pool(name="ffw", bufs=2))
    ps_sc = es.enter_context(tc.tile_pool(name="ps_sc", bufs=4, space="PSUM"))
    ps_g = es.enter_context(tc.tile_pool(name="ps_g", bufs=2, space="PSUM"))
    ps_ctx = es.enter_context(tc.tile_pool(name="ps_ctx", bufs=1, space="PSUM"))
    ps_s = es.enter_context(tc.tile_pool(name="ps_s", bufs=1, space="PSUM"))
    dp = es.enter_context(tc.tile_pool(name="dram", bufs=2, space="DRAM"))

    # ---- persistent SBUF ----
    At = cp.tile([P, KT, SBLK], F32)
    xs = cp.tile([P, KT, 2], F32)
    hN = cp.tile([P, KT, EMB], F32)
    hTfull = cp.tile([33, N], F32)
    hTown = cp.tile([33, SBLK], F32)
    U_aug = cp.tile([33, SBLK], F32)
    U0_aug = cp.tile([3, SBLK], F32)
    x2_aug = cp.tile([33, SBLK], F32)
    Karr = cp.tile([P, N], F32)
    Varr = cp.tile([P, KT, 36], F32)
    Qm = cp.tile([P, HEADS, SBLK], F32)
    w3t = cp.tile([3, EMB], F32)
    gwt = cp.tile([33, NCONV, EMB], F32)
    qwt = cp.tile([33, NDEC, 128], F32)
    kwt = cp.tile([33, NDEC, 128], F32)
    vwt = cp.tile([33, NDEC, 36], F32)
    wot = cp.tile([P, NDEC, 32], F32)
    e128t = cp.tile([P, 128], F32)
    f2bt = cp.tile([1, NDEC, EMB], F32)
    lnwt = cp.tile([EMB, 4, NDEC], F32)
    fcat = cp.tile([33, 2], F32)
    ident32 = cp.tile([32, 32], F32)
    ones32inv = cp.tile([32, 1], F32)
    ones1_32 = cp.tile([1, 32], F32)
    ones_row = cp.tile([1, SBLK], F32)
    epsA = cp.tile([1, 1], F32)

    # ---- stage 0: loads + const init ----
    for kt in range(KT):
        nc.sync.dma_start(out=At[:, kt, :], in_=d_at.ap()[P * kt:P * (kt + 1), :])
    nc.sync.dma_start(out=xs[:], in_=d_x.ap().rearrange("(k p) e -> p k e", k=KT))
    nc.sync.dma_start(out=U0_aug[2:3, :], in_=d_a1.ap()[None, :])
    nc.sync.dma_start(out=w3t[:], in_=d_w3.ap())
    nc.sync.dma_start(out=gwt[:], in_=d_gw.ap().rearrange("i r e -> r i e"))
    nc.sync.dma_start(out=qwt[:], in_=d_qw.ap().rearrange("l r e -> r l e"))
    nc.sync.dma_start(out=kwt[:], in_=d_kw.ap().rearrange("l r e -> r l e"))
    nc.sync.dma_start(out=vwt[:], in_=d_vw.ap().rearrange("l r e -> r l e"))
    nc.sync.dma_start(out=wot[:], in_=d_wo.ap().rearrange("l r e -> r l e"))
    nc.sync.dma_start(out=e128t[:], in_=d_e128.ap())
    nc.sync.dma_start(out=f2bt[:], in_=d_f2b.ap()[None, :, :])
    nc.sync.dma_start(out=lnwt[:], in_=d_lnw.ap())
    nc.sync.dma_start(out=fcat[:], in_=d_fca.ap())
    make_identity(nc, ident32[:])
    nc.vector.memset(ones32inv[:], 1.0 / 32.0)
    nc.vector.memset(ones1_32[:], 1.0)
    nc.vector.memset(ones_row[:], 1.0)
    nc.vector.memset(epsA[:], 1e-5)
    nc.vector.memset(U_aug[32:33, :], 1.0)
    nc.vector.memset(x2_aug[32:33, :], 1.0)
    nc.vector.memset(hTown[32:33, :], 1.0)
    nc.vector.memset(hTfull[32:33, :], 1.0)
    nc.vector.memset(Qm[:], 0.0)

    def ag_normal():
        """hTown[0:32] -> 4 transposes -> AG -> hN full."""
        hNo = wp.tile([P, 4, EMB], F32, tag="hNo", bufs=2)
        for k in range(4):
            tp = ps_s.tile([P, EMB], F32, tag="s")
            nc.tensor.transpose(tp[:], hTown[0:32, P * k:P * (k + 1)], ident32[:])
            nc.vector.tensor_copy(hNo[:, k, :], tp[:])
        agi = dp.tile([SBLK, EMB], F32, tag="agNi")
        ago = dp.tile([N, EMB], F32, tag="agNo")
        nc.sync.dma_start(out=agi[:].rearrange("(k p) e -> p k e", k=4), in_=hNo[:])
        nc.gpsimd.collective_compute("AllGather", mybir.AluOpType.bypass,
                                     replica_groups=RG, ins=[agi.opt()], outs=[ago.opt()])
        agov = ago[:].rearrange("(k p) e -> p k e", k=KT)
        for g in range(4):
            nc.sync.dma_start(out=hN[:, 8 * g:8 * (g + 1), :], in_=agov[:, 8 * g:8 * (g + 1), :])

    def ag_transposed():
        """hTown[0:32] -> AG -> hTfull[0:32]."""
        agi = dp.tile([32, SBLK], F32, tag="agTi")
        ago = dp.tile([NC * 32, SBLK], F32, tag="agTo")
        nc.sync.dma_start(out=agi[:], in_=hTown[0:32, :])
        nc.gpsimd.collective_compute("AllGather", mybir.AluOpType.bypass,
                                     replica_groups=RG, ins=[agi.opt()], outs=[ago.opt()])
        srcv = ago[:].rearrange("(c e) s -> e c s", c=NC)
        dstv = hTfull[0:32, :].rearrange("e (c s) -> e c s", c=NC)
        for g in range(2):
            nc.sync.dma_start(out=dstv[:, 4 * g:4 * (g + 1), :], in_=srcv[:, 4 * g:4 * (g + 1), :])

    # ---- GCN layer 1 (embed folded) ----
    p0 = ps_s.tile([2, SBLK], F32, tag="s")
    for kt in range(KT):
        nc.tensor.matmul(p0[:], xs[:, kt, :], At[:, kt, :], start=(kt == 0), stop=(kt == KT - 1))
    nc.vector.tensor_copy(U0_aug[0:2, :], p0[:])
    u1 = ps_s.tile([EMB, SBLK], F32, tag="s")
    nc.tensor.matmul(u1[:], w3t[:], U0_aug[:], start=True, stop=True)
    nc.vector.tensor_copy(U_aug[0:32, :], u1[:])
    z1 = ps_s.tile([EMB, SBLK], F32, tag="s")
    nc.tensor.matmul(z1[:], gwt[:, 0, :], U_aug[:], start=True, stop=True)
    nc.scalar.activation(hTown[0:32, :], z1[:], AF.Relu)
    ag_normal()
    if STAGE == 1:
        nc.sync.dma_start(out=d_dbgA.ap()[:, 0:KT * EMB],
                          in_=hN[:].rearrange("p k e -> p (k e)"))
        es.close()
        return

    # ---- GCN layers 2..9 ----
    for i in range(1, NCONV):
        u = ps_s.tile([EMB, SBLK], F32, tag="s")
        for kt in range(KT):
            nc.tensor.matmul(u[:], hN[:, kt, :], At[:, kt, :], start=(kt == 0), stop=(kt == KT - 1))
        nc.vector.tensor_copy(U_aug[0:32, :], u[:])
        z = ps_s.tile([EMB, SBLK], F32, tag="s")
        nc.tensor.matmul(z[:], gwt[:, i, :], U_aug[:], start=True, stop=True)
        nc.scalar.activation(hTown[0:32, :], z[:], AF.Relu)
        if i < NCONV - 1:
            ag_normal()
        else:
            ag_transposed()
    if STAGE == 2:
        if os.environ.get("KDBG") == "1":
            nc.sync.dma_start(out=d_dbgB.ap(), in_=hTfull[:])
        es.close()
        return

    # ---- LayerNorm helper (transposed layout) ----
    def layer_norm(res_psum, add_sbuf, w_ap, b_ap, out_ap):
        xsq = wp.tile([32, 2 * SBLK], F32, tag="xsq")
        nc.vector.tensor_add(xsq[:, 0:SBLK], res_psum, add_sbuf)
        nc.vector.tensor_mul(xsq[:, SBLK:], xsq[:, 0:SBLK], xsq[:, 0:SBLK])
        stats = wp.tile([1, 2 * SBLK], F32, tag="stats")
        st_a = ps_s.tile([1, SBLK], F32, tag="s")
        nc.tensor.matmul(st_a[:], ones32inv[:], xsq[:, 0:SBLK], start=True, stop=True)
        nc.vector.tensor_copy(stats[:, 0:SBLK], st_a[:])
        st_b = ps_s.tile([1, SBLK], F32, tag="s")
        nc.tensor.matmul(st_b[:], ones32inv[:], xsq[:, SBLK:], start=True, stop=True)
        nc.vector.tensor_copy(stats[:, SBLK:], st_b[:])
        veps = wp.tile([1, SBLK], F32, tag="veps")
        # veps = meansq - mean^2 + eps
        m2 = wp.tile([1, SBLK], F32, tag="m2")
        nc.vector.tensor_mul(m2[:], stats[:, 0:SBLK], stats[:, 0:SBLK])
        nc.vector.tensor_sub(veps[:], stats[:, SBLK:], m2[:])
        nc.vector.tensor_scalar(veps[:], veps[:], 1e-5, None, mybir.AluOpType.add)
        lnv = wp.tile([1, SBLK], F32, tag="lnv")
        nc.scalar.activation(lnv[:], veps[:], AF.Ln)
        y0 = wp.tile([1, SBLK], F32, tag="y0")
        nc.scalar.activation(y0[:], lnv[:], AF.Exp, scale=-0.5)
        # one Newton step: y1 = y0 * (1.5 - 0.5 * veps * y0^2)
        nt = wp.tile([1, SBLK], F32, tag="nt")
        nc.vector.tensor_mul(nt[:], y0[:], y0[:])
        nc.vector.tensor_mul(nt[:], nt[:], veps[:])
        nc.vector.tensor_scalar(nt[:], nt[:], -0.5, 1.5, mybir.AluOpType.mult, mybir.AluOpType.add)
        iq = wp.tile([1, 2 * SBLK], F32, tag="iq")
        nc.vector.tensor_mul(iq[:, 0:SBLK], y0[:], nt[:])
        nc.vector.tensor_mul(iq[:, SBLK:], stats[:, 0:SBLK], iq[:, 0:SBLK])
        rep2a = ps_s.tile([32, SBLK], F32, tag="s")
        nc.tensor.matmul(rep2a[:], ones1_32[:], iq[:, 0:SBLK], start=True, stop=True)
        t1 = wp.tile([32, SBLK], F32, tag="t1")
        nc.vector.tensor_mul(t1[:], xsq[:, 0:SBLK], rep2a[:])
        rep2b = ps_s.tile([32, SBLK], F32, tag="s")
        nc.tensor.matmul(rep2b[:], ones1_32[:], iq[:, SBLK:], start=True, stop=True)
        nc.vector.tensor_sub(t1[:], t1[:], rep2b[:])
        nc.vector.tensor_scalar(out_ap, t1[:], w_ap, b_ap, mybir.AluOpType.mult, mybir.AluOpType.add)

    # ---- transformer layers ----
    for l in range(NDEC):
        # K_arr
        for j in range(8):
            pk = ps_g.tile([P, SBLK], F32, tag="pg")
            nc.tensor.matmul(pk[:], kwt[:, l, :], hTfull[:, SBLK * j:SBLK * (j + 1)], start=True, stop=True)
            nc.vector.tensor_copy(Karr[:, SBLK * j:SBLK * (j + 1)], pk[:])
        # Q + masked per-head copies
        pq = ps_g.tile([P, SBLK], F32, tag="pg")
        nc.tensor.matmul(pq[:], qwt[:, l, :], hTown[:], start=True, stop=True)
        for h in range(HEADS):
            nc.vector.tensor_copy(Qm[32 * h:32 * h + 8, h, :], pq[32 * h:32 * h + 8, :])
        # V_arr (ones column generated via vw aug row)
        for g in range(KT // 4):
            pv = ps_s.tile([P, 4, 36], F32, tag="s")
            for q in range(4):
                kt = 4 * g + q
                nc.tensor.matmul(pv[:, q, :], hTfull[:, P * kt:P * (kt + 1)], vwt[:, l, :],
                                 start=True, stop=True)
            nc.vector.tensor_copy(Varr[:, 4 * g:4 * (g + 1), :], pv[:])
        if STAGE == 3 and l == 0:
            nc.sync.dma_start(out=d_dbgA.ap(), in_=Karr[:])
            nc.sync.dma_start(out=d_dbgB.ap()[0:33, 0:SBLK], in_=hTown[:])
            es.close()
            return
        # flash loop
        ctx = ps_ctx.tile([P, SBLK], F32, tag="ctx")
        nc.vector.memset(ctx[:], 0.0)
        for kt in range(KT):
            for half in range(2):
                S = ps_sc.tile([P, 2 * SBLK], F32, tag="S", bufs=2)
                for hh in range(2):
                    h = 2 * half + hh
                    nc.tensor.matmul(S[:, SBLK * hh:SBLK * (hh + 1)],
                                     Karr[:, P * kt:P * (kt + 1)], Qm[:, h, :],
                                     start=True, stop=True)
                E = ep.tile([P, 2 * SBLK], F32, tag="E", bufs=3)
                nc.scalar.activation(E[:], S[:], AF.Exp, scale=SCALE)
                for hh in range(2):
                    h = 2 * half + hh
                    nc.tensor.matmul(ctx[32 * h:32 * h + 9, :], Varr[:, kt, 9 * h:9 * h + 9],
                                     E[:, SBLK * hh:SBLK * (hh + 1)],
                                     start=(kt == 0), stop=(kt == KT - 1),
                                     tile_position=(0, 32 * h))
        if STAGE == 4 and l == 0:
            ctd = gp.tile([P, SBLK], F32, tag="ctd", bufs=1)
            nc.vector.tensor_copy(ctd[:], ctx[:])
            nc.sync.dma_start(out=d_dbgA.ap()[:, 0:SBLK], in_=ctd[:])
            es.close()
            return
        # softmax denominators + out-projection
        cte = gp.tile([P, SBLK], F32, tag="cte", bufs=1)
        nc.vector.tensor_scalar(cte[:], ctx[:], 1e-30, None, mybir.AluOpType.add)
        rcp = gp.tile([P, SBLK], F32, tag="rcp", bufs=1)
        nc.vector.reciprocal(rcp[:], cte[:])
        rep = ps_g.tile([P, SBLK], F32, tag="pg")
        nc.tensor.matmul(rep[:], e128t[:], rcp[:], start=True, stop=True)
        ctn = gp.tile([P, SBLK], F32, tag="ctn", bufs=1)
        nc.vector.tensor_mul(ctn[:], cte[:], rep[:])
        attn = ps_s.tile([32, SBLK], F32, tag="s")
        nc.tensor.matmul(attn[:], wot[:, l, :], ctn[:], start=True, stop=True)
        # LN1 -> x2_aug
        layer_norm(attn[:], hTown[0:32, :], lnwt[:, 0, l:l + 1], lnwt[:, 1, l:l + 1],
                   x2_aug[0:32, :])
        if STAGE == 5 and l == 0:
            nc.sync.dma_start(out=d_dbgB.ap()[0:33, 0:SBLK], in_=x2_aug[:])
            es.close()
            return
        # FFN
        f1t = ffp.tile([33, FF], F32, tag="f1")
        nc.sync.dma_start(out=f1t[:], in_=d_f1.ap()[l])
        f2t = ffp.tile([P, FF // P, EMB], F32, tag="f2")
        nc.sync.dma_start(out=f2t[:], in_=d_f2.ap()[l].rearrange("(t p) e -> p t e", p=P))
        y = ps_s.tile([EMB, SBLK], F32, tag="s")
        for ft in range(FF // P):
            g_ps = ps_g.tile([P, SBLK], F32, tag="pg")
            nc.tensor.matmul(g_ps[:], f1t[:, P * ft:P * (ft + 1)], x2_aug[:], start=True, stop=True)
            g_sb = gp.tile([P, SBLK], F32, tag="g")
            if ft % 2 == 0:
                nc.scalar.activation(g_sb[:], g_ps[:], AF.Relu)
            else:
                nc.vector.tensor_scalar(g_sb[:], g_ps[:], 0.0, None, mybir.AluOpType.max)
            nc.tensor.matmul(y[:], f2t[:, ft, :], g_sb[:], start=(ft == 0), stop=False)
        nc.tensor.matmul(y[:], f2bt[:, l, :], ones_row[:], start=False, stop=True)
        # LN2 -> hTown
        layer_norm(y[:], x2_aug[0:32, :], lnwt[:, 2, l:l + 1], lnwt[:, 3, l:l + 1],
                   hTown[0:32, :])
        if STAGE == 6 and l == 0:
            nc.sync.dma_start(out=d_dbgB.ap()[0:33, 0:SBLK], in_=hTown[:])
            es.close()
            return
        if l < NDEC - 1:
            ag_transposed()

    # ---- pooling + fc ----
    red = wp.tile([32, 1], F32, tag="red")
    nc.vector.reduce_sum(red[:], hTown[0:32, :], axis=mybir.AxisListType.X)
    po = wp.tile([32, 1], F32, tag="po")
    nc.vector.tensor_scalar(po[:], red[:], 1.0 / N, None, mybir.AluOpType.mult)
    agi = dp.tile([32, 1], F32, tag="agPi")
    ago = dp.tile([NC * 32, 1], F32, tag="agPo")
    nc.sync.dma_start(out=agi[:], in_=po[:])
    nc.gpsimd.collective_compute("AllGather", mybir.AluOpType.bypass,
                                 replica_groups=RG, ins=[agi.opt()], outs=[ago.opt()])
    pool8 = wp.tile([32, NC], F32, tag="pool8")
    nc.sync.dma_start(out=pool8[:], in_=ago[:].rearrange("(c e) o -> e (c o)", c=NC))
    pa = cp.tile([33, 1], F32)
    nc.vector.memset(pa[32:33, :], 1.0)
    nc.vector.reduce_sum(pa[0:32, :], pool8[:], axis=mybir.AxisListType.X)
    op = ps_s.tile([1, 2], F32, tag="s")
    nc.tensor.matmul(op[:], pa[:], fcat[:], start=True, stop=True)
    osb = wp.tile([1, 2], F32, tag="osb")
    nc.vector.tensor_copy(osb[:], op[:])
    nc.sync.dma_start(out=d_out.ap(), in_=osb[:])
    es.close()


_CACHE = {}


def _get_program():
    import os
    key = "nc" + os.environ.get("KSTAGE", "99") + os.environ.get("KDBG", "0")
    if key in _CACHE:
        return _CACHE[key]
    import concourse.bass as bass
    import concourse.mybir as mybir
    import concourse.tile as tile
    from concourse import bacc
    from concourse.masks import make_identity

    nc = bacc.Bacc("TRN2", target_bir_lowering=False, debug=False, num_devices=NC)
    with tile.TileContext(nc) as tc:
        _build(nc, tc, tile, mybir, bass, make_identity)
    nc.compile()
    _CACHE[key] = nc
    return nc


def _get_runner():
    """Cached shard_map executable over 8 cores (modeled on run_bass_via_pjrt)."""
    if "runner" in _CACHE:
        return _CACHE["runner"]
    import jax
    globals()["jax"] = jax
    import concourse.mybir as mybir
    from concourse import bass2jax

    nc = _get_program()
    bass2jax.install_neuronx_cc_hook()

    part_name = nc.partition_id_tensor.name if nc.partition_id_tensor else None
    in_names, out_names, out_avals, zero_outs = [], [], [], []
    for alloc in nc.m.functions[0].allocations:
        if not isinstance(alloc, mybir.MemoryLocationSet):
            continue
        name = alloc.memorylocations[0].name
        if alloc.kind == "ExternalInput":
            if name != part_name:
                in_names.append(name)
        elif alloc.kind == "ExternalOutput":
            shape = tuple(alloc.tensor_shape)
            dtype = mybir.dt.np(alloc.dtype)
            out_names.append(name)
            out_avals.append(jax.core.ShapedArray(shape, dtype))
            zero_outs.append(np.zeros(shape, dtype))
    n_params = len(in_names)
    all_names = in_names + out_names
    if part_name is not None:
        all_names = all_names + [part_name]

    def _body(*args):
        operands = list(args)
        if part_name is not None:
            operands.append(bass2jax.partition_id_tensor())
        outs = bass2jax._bass_exec_p.bind(
            *operands,
            out_avals=tuple(out_avals),
            in_names=tuple(all_names),
            out_names=tuple(out_names),
            lowering_input_output_aliases=(),
            sim_require_finite=True,
            sim_require_nnan=True,
            nc=nc,
        )
        return tuple(outs)

    devices = jax.devices()[:NC]
    mesh = bass2jax.Mesh(np.asarray(devices), ("core",))
    n_outs = len(out_names)
    sharded = jax.jit(
        bass2jax.shard_map(
            _body, mesh=mesh,
            in_specs=(bass2jax.PartitionSpec("core"),) * (n_params + n_outs),
            out_specs=(bass2jax.PartitionSpec("core"),) * n_outs,
            check_rep=False,
        ),
        donate_argnums=tuple(range(n_params, n_params + n_outs)),
        keep_unused=True,
    )

    from jax.sharding import NamedSharding, PartitionSpec as PS
    shard = NamedSharding(mesh, PS("core"))

    def _stage(shared, per_core, dev_key):
        concat_in = []
        for nm in in_names:
            if nm in per_core:
                concat_in.append(np.ascontiguousarray(per_core[nm]))
            else:
                a = np.ascontiguousarray(shared[nm])
                concat_in.append(np.broadcast_to(a, (NC, *a.shape)).reshape(NC * a.shape[0], *a.shape[1:]))
        dev_arrs = [jax.device_put(a, shard) for a in concat_in]
        for a in dev_arrs:
            a.block_until_ready()
        dev = (dev_key, dev_arrs)
        _CACHE["dev_in"] = dev
        return dev

    def run(shared, per_core):
        import time as _time
        dev_key = ("dev", id(shared), id(per_core))
        dev = _CACHE.get("dev_in")
        if dev is None or dev[0] != dev_key:
            dev = _stage(shared, per_core, dev_key)
        last_exc = None
        for attempt in range(5):
            try:
                concat_zeros = [np.zeros((NC * z.shape[0], *z.shape[1:]), z.dtype) for z in zero_outs]
                out_arrs = sharded(*dev[1], *concat_zeros)
                return {
                    nm: np.asarray(out_arrs[i]).reshape(NC, *out_avals[i].shape)[0]
                    for i, nm in enumerate(out_names)
                }
            except Exception as e:  # transient device-unrecoverable after aborted runs
                last_exc = e
                _time.sleep(4.0 * (attempt + 1))
                dev = _stage(shared, per_core, dev_key)
        raise last_exc

    _CACHE["runner"] = run
    _CACHE["sharded_fn"] = sharded
    return run


def _input_key(inp):
    import hashlib
    hsh = hashlib.sha256()
    for k in sorted(inp):
        hsh.update(k.encode())
        hsh.update(np.ascontiguousarray(inp[k]).tobytes())
    return hsh.hexdigest()


def kernel(**inputs):
    inp = {k: np.asarray(v) for k, v in inputs.items()}
    key = _input_key(inp)
    run = _get_runner()
    cached = _CACHE.get("staged")
    if cached is None or cached[0] != key:
        pre = _host_prep(inp)
        shared = {
            "x": np.ascontiguousarray(inp["x"], np.float32),
            "w3": pre["w3"], "gw": pre["gw"], "qw": pre["qw"], "kw": pre["kw"],
            "vw": pre["vw"], "wo": pre["wo"], "e128": pre["E128"], "f1": pre["f1"],
            "f2": pre["f2"], "f2b": pre["f2b"], "lnw": pre["lnw"], "fca": pre["fca"],
        }
        per_core = {
            "a_t": pre["AT3"].reshape(NC * N, SBLK),
            "a1": pre["a1"].reshape(NC, SBLK).reshape(NC * SBLK),
        }
        _CACHE["staged"] = (key, shared, per_core)
    else:
        _, shared, per_core = cached

    outs = run(shared, per_core)
    kernel.last_outs = outs
    return outs["out"]


if __name__ == "__main__":
    import test as T
    T.main()
